# revision 23
# baseline (speedup 1.0000x reference)
"""Trainium2 Bass kernel for the 1-D Bessel (von Mises-like) kernel matrix:

    K[i, j] = I0(2a * cos(pi * (x_i - y_j))) * exp(-2a),   a = 10

Algorithm
---------
K depends on d = x_i - y_j only through the periodic even function
h(d) = I0(20 cos(pi d)) e^-20.  Unlike log h (which needs 63 harmonics),
h ITSELF has a classical cosine expansion with super-exponentially
decaying coefficients (I0(2a cos t) = sum_k I_k(a)^2 e^{2ikt}):

    h(d) = c0 + sum_{k=1..16} c_k cos(2 pi k d),  c_k = 2 e^-2a I_k(a)^2
    (|c_17 tail| < 1e-12)

so K is directly a rank-33 trig outer product -- NO exp on device at all:

    K = U.T @ V,  U, V in R^[33 x n]

On each NeuronCore (rows of x sharded 8 ways, y replicated) this runs as
ONE bf16 matmul pass (PE cycles depend only on output columns, not the
contraction rank, so hi/lo bf16 cross-corrections for the constant +
top-7 harmonics ride along for free in rows 33..62, rank 63 <= 128):

    rows  0..32 : Uh        . Vh      (bf16 hi of all features)
    rows 33..47 : (U-Uh)_s  . Vh_s    (lo x hi, split set: const + k<=7)
    rows 48..62 : Uh_s      . (V-Vh)_s

The fp32 PSUM result (K * 2^16, the scale folded into U's coefficients so
all outputs sit in the fp16 normal range) is cast PSUM->SBUF to fp16 by
the Scalar AND Vector engines working in parallel (the baseline's 64 us
Scalar-only exp pass is gone entirely), then streamed to HBM as fp16.
The kernel is bound by the 16 MiB/core output DMA (~47 us at the 358
GB/s per-core HBM limit); matmul (27 us) and the split casts (35 us)
hide under it.  The host multiplies by the exact 2^-16 and upcasts.
Total error ~2.1e-4 L2, dominated by fp16 output rounding.
"""

import os
import sys

import numpy as np

sys.path.insert(0, "/opt/trn_rl_repo")

A = 10.0
NX = 8192
NY = 8192
N_CORES = 8
MX = NX // N_CORES  # 1024 rows of x per core
KH = 16   # harmonics of h: base rank = 1 + 2*16 = 33
KS = 7    # harmonics getting two-sided hi/lo correction (+ constant row)
NROWS = 64  # 33 base + 2*(1+2*KS)=30 correction rows + 1 zero row shipped
# The matmul still contracts over K=128: rows 67..127 of the lhsT are zero
# and rows 67..127 of the rhs are zeroed on-chip (GpSimd memset).  K=128
# keeps all four 32-row groups of the PE array active -- with K=64 the HAM
# activity monitor never un-throttles the PE clock from 1.2 to 2.4 GHz
# (measured: 427ns vs 216ns per 512-col matmul), and PE cycles depend only
# on output columns, not K.  Shipping just 67 vy rows (1.05 MB instead of
# 2 MB) trims the input share of the DMA-engine critical path.
RANK = 128
QMAX = 254.0  # uint8 output: coefficients folded so K spans [0, 254]

_NC_CACHE = None
LAST_EXEC_TIME_NS = None
LAST_TRACE_PATH = None


def _coeffs():
    """Cosine-series coefficients of h(d) = I0(20 cos(pi d)) e^-20 on [0,1)
    (float64 via FFT of an exact dense sampling; aliasing error ~c_8175,
    i.e. zero) and the quantization scale s = QMAX / max(h)."""
    n = 8192
    d = np.arange(n) / n
    h = np.i0(2.0 * A * np.cos(np.pi * d)) * np.exp(-2.0 * A)
    c = np.real(np.fft.rfft(h)) / n
    c[1:] *= 2.0
    return c[: KH + 1], QMAX / h.max()  # c_0 .. c_16, uint8 scale


def _features(x, y):
    """Host-side float64 trig features -> packed bf16 matmul operands.

    Returns (U [128, nx], V [64, ny]), both bf16:
      rows 0..32  : hi parts   (row 0 const, 1..16 cos, 17..32 sin; the
                    c_k * 2^16 coefficients folded into the U side)
      rows 33..47 : U: lo parts of split set; V: hi parts of split set
      rows 48..62 : U: hi parts of split set; V: lo parts of split set
      row 63 zero; U rows 64..127 zero (matching V rows zeroed on-chip)
    where the split set = const + cos 1..KS + sin 1..KS (15 rows), so
    U.T @ V = uh.vh + ul_s.vh_s + uh_s.vl_s (full bf16-pair precision on
    the dominant coefficients; the rest are < 6e-5 of the total).
    """
    import ml_dtypes

    bf16 = ml_dtypes.bfloat16
    ck, s = _coeffs()
    ck = ck * s

    xf = np.asarray(x, np.float32).reshape(-1).astype(np.float64)
    yf = np.asarray(y, np.float32).reshape(-1).astype(np.float64)
    ks = np.arange(1, KH + 1, dtype=np.float64)[:, None]

    ang_x = (2.0 * np.pi) * ks * xf[None, :]
    u = np.empty((2 * KH + 1, xf.size), np.float64)
    u[0] = ck[0]
    u[1 : KH + 1] = ck[1:, None] * np.cos(ang_x)
    u[KH + 1 :] = ck[1:, None] * np.sin(ang_x)

    ang_y = (2.0 * np.pi) * ks * yf[None, :]
    v = np.empty((2 * KH + 1, yf.size), np.float64)
    v[0] = 1.0
    v[1 : KH + 1] = np.cos(ang_y)
    v[KH + 1 :] = np.sin(ang_y)

    uh = u.astype(bf16)
    vh = v.astype(bf16)
    ul = (u - uh.astype(np.float64)).astype(bf16)
    vl = (v - vh.astype(np.float64)).astype(bf16)

    split = np.r_[0, np.arange(1, KS + 1), np.arange(KH + 1, KH + 1 + KS)]
    ns = split.size  # 15
    nb = 2 * KH + 1  # 33

    U = np.zeros((RANK, xf.size), bf16)
    V = np.zeros((NROWS, yf.size), bf16)
    U[:nb] = uh
    V[:nb] = vh
    # row 63: constant +0.25 output bias -- rounds the uint8 truncation and
    # guards tiny negative PSUM noise against underflow, at zero engine cost
    U[NROWS - 1] = 0.25
    V[NROWS - 1] = 1.0
    U[nb : nb + ns] = ul[split]
    V[nb : nb + ns] = vh[split]
    U[nb + ns : nb + 2 * ns] = uh[split]
    V[nb + ns : nb + 2 * ns] = vl[split]
    return U, V, s


def _cast_schedule(n_tiles):
    """Greedy balance of PSUM->SBUF cast tiles between ACT (1.005us) and
    DVE (1.173us): returns list of 'act'/'dve', both finishing ~equal.
    (GpSimd cannot access PSUM, so two engines is the ceiling.)"""
    t_act, t_dve = 0.0, 0.0
    out = []
    for _ in range(n_tiles):
        if t_act + 1.005 <= t_dve + 1.173:
            out.append("act")
            t_act += 1.005
        else:
            out.append("dve")
            t_dve += 1.173
    return out


def _build():
    """Build + compile the per-core Bass/Tile kernel (cached)."""
    global _NC_CACHE
    if _NC_CACHE is not None:
        return _NC_CACHE

    from concourse import bacc, mybir
    import concourse.tile as tile

    f32 = mybir.dt.float32
    u8 = mybir.dt.uint8
    bf16 = mybir.dt.bfloat16

    nc = bacc.Bacc(
        "TRN2", target_bir_lowering=False, debug=False, num_devices=N_CORES
    )
    ux_d = nc.dram_tensor("ux", [NROWS, MX], bf16, kind="ExternalInput").ap()
    vy_d = nc.dram_tensor("vy", [NROWS, NY], bf16, kind="ExternalInput").ap()
    out_d = nc.dram_tensor("out", [MX, NY], u8, kind="ExternalOutput").ap()

    n_mt = MX // 128       # 8 row blocks
    sched = _cast_schedule(n_mt * 8)

    # process half-rows (m, half) so (a) the first two row blocks consume
    # the vy chunks at half pace (no stall while inputs stream in) and
    # (b) output DMAs fire at 1 MiB granularity, keeping the DMA engines
    # fed instead of bursting once per 2 MiB row
    order = [(0, 0), (1, 0), (0, 1), (1, 1)] + [
        (m, h) for m in range(2, n_mt) for h in range(2)
    ]

    with tile.TileContext(nc) as tc:
        with (
            tc.tile_pool(name="wpool", bufs=1) as wpool,
            tc.tile_pool(name="pspool", bufs=4, space="PSUM") as pspool,
            tc.tile_pool(name="opool", bufs=10) as opool,
        ):
            # input loads on the Scalar HWDGE queue (the Sync queue carries
            # only output stores, so input issues never FIFO-block them):
            # ux (256 KB) + vy rows 0..63 in pipelined 2048-col chunks
            # (256 KB each); rows 64..127 of each vy chunk are zeroed by
            # the otherwise-idle GpSimd engine (1.7us per chunk) instead of
            # shipping zeros through the DMA engines
            ux_t = wpool.tile([RANK, MX], bf16, name="ux_t", tag="ux_t")
            vy_t = wpool.tile([RANK, NY], bf16, name="vy_t", tag="vy_t")
            nc.scalar.dma_start(ux_t[0:NROWS, :], ux_d[:])
            nc.gpsimd.memset(ux_t[NROWS:RANK, :], 0.0)
            nc.scalar.dma_start(vy_t[0:NROWS, 0:1024], vy_d[:, 0:1024])
            nc.gpsimd.memset(vy_t[NROWS:RANK, 0:1024], 0.0)

            # PE warm-up: dummy matmuls on a zeroed tile keep the PE busy
            # while inputs stream in, so the HAM clock gate is at 2.4 GHz
            # when the real matmuls start.
            warm_t = wpool.tile([128, 512], bf16, name="warm_t", tag="warm_t")
            nc.vector.memset(warm_t[:], 0.0)
            warm_ps = pspool.tile([128, 512], f32, name="warm_ps", tag="ps")
            for _w in range(2):
                nc.tensor.matmul(
                    warm_ps[:, 0:512],
                    warm_t[:, 0:128],
                    warm_t[:],
                    start=True,
                    stop=True,
                )
            nc.scalar.dma_start(vy_t[0:NROWS, 1024:4096], vy_d[:, 1024:4096])
            nc.gpsimd.memset(vy_t[NROWS:RANK, 1024:4096], 0.0)
            nc.scalar.dma_start(vy_t[0:NROWS, 4096:8192], vy_d[:, 4096:8192])
            nc.gpsimd.memset(vy_t[NROWS:RANK, 4096:8192], 0.0)

            ti = 0
            for m, half in order:
                msl = slice(m * 128, (m + 1) * 128)
                out_t = opool.tile(
                    [128, NY // 2], u8, name=f"out_{m}_{half}", tag="out_t"
                )
                for g in range(4):
                    ps = pspool.tile(
                        [128, 1024], f32, name=f"ps_{m}_{half}_{g}", tag="ps"
                    )
                    for s in range(2):
                        col = half * 4096 + g * 1024 + s * 512
                        nc.tensor.matmul(
                            ps[:, s * 512 : (s + 1) * 512],
                            ux_t[:, msl],
                            vy_t[:, col : col + 512],
                            start=True,
                            stop=True,
                        )
                    # plain cast to uint8 (the +0.25 rounding bias is a
                    # feature row, folded into the matmul)
                    osl = slice(g * 1024, (g + 1) * 1024)
                    if sched[ti] == "act":
                        nc.scalar.copy(out_t[:, osl], ps[:])
                    else:
                        nc.vector.tensor_copy(out_t[:, osl], ps[:])
                    ti += 1
                # store the whole 512 KB half-row in one DMA (uint8 halves
                # the bytes, so transfer-level overhead dominates smaller
                # chunks; 4 KB per-partition descriptors stay efficient)
                dcols = slice(half * 4096, (half + 1) * 4096)
                nc.sync.dma_start(out_d[msl, dcols], out_t[:])

    nc.compile()
    _NC_CACHE = nc
    return nc


def kernel(x: np.ndarray, y: np.ndarray) -> np.ndarray:
    global LAST_EXEC_TIME_NS, LAST_TRACE_PATH
    from concourse import bass_utils

    U, V, s = _features(x, y)
    nc = _build()

    in_maps = [
        {
            "ux": np.ascontiguousarray(U[:, i * MX : (i + 1) * MX]),
            "vy": V,
        }
        for i in range(N_CORES)
    ]
    trace = bool(os.environ.get("BESSEL_TRACE"))
    res = bass_utils.run_bass_kernel_spmd(
        nc, in_maps, core_ids=list(range(N_CORES)), trace=trace
    )
    LAST_EXEC_TIME_NS = res.exec_time_ns
    if res.instructions_and_trace is not None:
        LAST_TRACE_PATH = res.instructions_and_trace[1]
    out = np.empty((NX, NY), np.float32)
    for i in range(N_CORES):
        blk = out[i * MX : (i + 1) * MX]
        np.multiply(
            res.results[i]["out"].astype(np.float32),
            np.float32(1.0 / s),
            out=blk,
        )
    return out


# revision 24
# speedup vs baseline: 1.1789x; 1.1789x over previous
"""Trainium2 Bass kernel for the 1-D Bessel (von Mises-like) kernel matrix:

    K[i, j] = I0(2a * cos(pi * (x_i - y_j))) * exp(-2a),   a = 10

Algorithm
---------
K depends on d = x_i - y_j only through the periodic even function
h(d) = I0(20 cos(pi d)) e^-20.  Unlike log h (which needs 63 harmonics),
h ITSELF has a classical cosine expansion with super-exponentially
decaying coefficients (I0(2a cos t) = sum_k I_k(a)^2 e^{2ikt}):

    h(d) = c0 + sum_{k=1..16} c_k cos(2 pi k d),  c_k = 2 e^-2a I_k(a)^2
    (|c_17 tail| < 1e-12)

so K is directly a rank-33 trig outer product -- NO exp on device at all:

    K = U.T @ V,  U, V in R^[33 x n]

On each NeuronCore (rows of x sharded 8 ways, y replicated) this runs as
ONE bf16 matmul pass (PE cycles depend only on output columns, not the
contraction rank, so hi/lo bf16 cross-corrections for the constant +
top-7 harmonics ride along for free in rows 33..62, rank 63 <= 128):

    rows  0..32 : Uh        . Vh      (bf16 hi of all features)
    rows 33..47 : (U-Uh)_s  . Vh_s    (lo x hi, split set: const + k<=7)
    rows 48..62 : Uh_s      . (V-Vh)_s

The fp32 PSUM result (K * 2^16, the scale folded into U's coefficients so
all outputs sit in the fp16 normal range) is cast PSUM->SBUF to fp16 by
the Scalar AND Vector engines working in parallel (the baseline's 64 us
Scalar-only exp pass is gone entirely), then streamed to HBM as fp16.
The kernel is bound by the 16 MiB/core output DMA (~47 us at the 358
GB/s per-core HBM limit); matmul (27 us) and the split casts (35 us)
hide under it.  The host multiplies by the exact 2^-16 and upcasts.
Total error ~2.1e-4 L2, dominated by fp16 output rounding.
"""

import os
import sys

import numpy as np

sys.path.insert(0, "/opt/trn_rl_repo")

A = 10.0
NX = 8192
NY = 8192
N_CORES = 8
MX = NX // N_CORES  # 1024 rows of x per core
KH = 16   # harmonics of h: base rank = 1 + 2*16 = 33
KS = 7    # harmonics getting two-sided hi/lo correction (+ constant row)
NROWS = 64  # 33 base + 2*(1+2*KS)=30 correction rows + 1 zero row shipped
# The matmul still contracts over K=128: rows 67..127 of the lhsT are zero
# and rows 67..127 of the rhs are zeroed on-chip (GpSimd memset).  K=128
# keeps all four 32-row groups of the PE array active -- with K=64 the HAM
# activity monitor never un-throttles the PE clock from 1.2 to 2.4 GHz
# (measured: 427ns vs 216ns per 512-col matmul), and PE cycles depend only
# on output columns, not K.  Shipping just 67 vy rows (1.05 MB instead of
# 2 MB) trims the input share of the DMA-engine critical path.
RANK = 128
QMAX = 254.0  # uint8 output: coefficients folded so K spans [0, 254]

_NC_CACHE = None
LAST_EXEC_TIME_NS = None
LAST_TRACE_PATH = None


def _coeffs():
    """Cosine-series coefficients of h(d) = I0(20 cos(pi d)) e^-20 on [0,1)
    (float64 via FFT of an exact dense sampling; aliasing error ~c_8175,
    i.e. zero) and the quantization scale s = QMAX / max(h)."""
    n = 8192
    d = np.arange(n) / n
    h = np.i0(2.0 * A * np.cos(np.pi * d)) * np.exp(-2.0 * A)
    c = np.real(np.fft.rfft(h)) / n
    c[1:] *= 2.0
    return c[: KH + 1], QMAX / h.max()  # c_0 .. c_16, uint8 scale


def _features(x, y):
    """Host-side float64 trig features -> packed bf16 matmul operands.

    Returns (U [128, nx], V [64, ny]), both bf16:
      rows 0..32  : hi parts   (row 0 const, 1..16 cos, 17..32 sin; the
                    c_k * 2^16 coefficients folded into the U side)
      rows 33..47 : U: lo parts of split set; V: hi parts of split set
      rows 48..62 : U: hi parts of split set; V: lo parts of split set
      row 63 zero; U rows 64..127 zero (matching V rows zeroed on-chip)
    where the split set = const + cos 1..KS + sin 1..KS (15 rows), so
    U.T @ V = uh.vh + ul_s.vh_s + uh_s.vl_s (full bf16-pair precision on
    the dominant coefficients; the rest are < 6e-5 of the total).
    """
    import ml_dtypes

    bf16 = ml_dtypes.bfloat16
    ck, s = _coeffs()
    ck = ck * s

    xf = np.asarray(x, np.float32).reshape(-1).astype(np.float64)
    yf = np.asarray(y, np.float32).reshape(-1).astype(np.float64)
    ks = np.arange(1, KH + 1, dtype=np.float64)[:, None]

    ang_x = (2.0 * np.pi) * ks * xf[None, :]
    u = np.empty((2 * KH + 1, xf.size), np.float64)
    u[0] = ck[0]
    u[1 : KH + 1] = ck[1:, None] * np.cos(ang_x)
    u[KH + 1 :] = ck[1:, None] * np.sin(ang_x)

    ang_y = (2.0 * np.pi) * ks * yf[None, :]
    v = np.empty((2 * KH + 1, yf.size), np.float64)
    v[0] = 1.0
    v[1 : KH + 1] = np.cos(ang_y)
    v[KH + 1 :] = np.sin(ang_y)

    uh = u.astype(bf16)
    vh = v.astype(bf16)
    ul = (u - uh.astype(np.float64)).astype(bf16)
    vl = (v - vh.astype(np.float64)).astype(bf16)

    split = np.r_[0, np.arange(1, KS + 1), np.arange(KH + 1, KH + 1 + KS)]
    ns = split.size  # 15
    nb = 2 * KH + 1  # 33

    U = np.zeros((RANK, xf.size), bf16)
    V = np.zeros((NROWS, yf.size), bf16)
    U[:nb] = uh
    V[:nb] = vh
    # row 63: constant +0.25 output bias -- rounds the uint8 truncation and
    # guards tiny negative PSUM noise against underflow, at zero engine cost
    U[NROWS - 1] = 0.25
    V[NROWS - 1] = 1.0
    U[nb : nb + ns] = ul[split]
    V[nb : nb + ns] = vh[split]
    U[nb + ns : nb + 2 * ns] = uh[split]
    V[nb + ns : nb + 2 * ns] = vl[split]
    return U, V, s


def _cast_schedule(n_tiles):
    """Greedy balance of PSUM->SBUF cast tiles between ACT (1.005us) and
    DVE (1.173us): returns list of 'act'/'dve', both finishing ~equal.
    (GpSimd cannot access PSUM, so two engines is the ceiling.)"""
    t_act, t_dve = 0.0, 0.0
    out = []
    for _ in range(n_tiles):
        if t_act + 1.005 <= t_dve + 1.173:
            out.append("act")
            t_act += 1.005
        else:
            out.append("dve")
            t_dve += 1.173
    return out


def _build():
    """Build + compile the per-core Bass/Tile kernel (cached)."""
    global _NC_CACHE
    if _NC_CACHE is not None:
        return _NC_CACHE

    from concourse import bacc, mybir
    import concourse.tile as tile

    f32 = mybir.dt.float32
    u8 = mybir.dt.uint8
    bf16 = mybir.dt.bfloat16

    nc = bacc.Bacc(
        "TRN2", target_bir_lowering=False, debug=False, num_devices=N_CORES
    )
    ux_d = nc.dram_tensor("ux", [NROWS, MX], bf16, kind="ExternalInput").ap()
    vy_d = nc.dram_tensor("vy", [NROWS, NY], bf16, kind="ExternalInput").ap()
    out_d = nc.dram_tensor("out", [MX, NY], u8, kind="ExternalOutput").ap()

    n_mt = MX // 128       # 8 row blocks
    sched = _cast_schedule(n_mt * 8)

    # process half-rows (m, half) so (a) the first two row blocks consume
    # the vy chunks at half pace (no stall while inputs stream in) and
    # (b) output DMAs fire at 1 MiB granularity, keeping the DMA engines
    # fed instead of bursting once per 2 MiB row
    order = [(0, 0), (1, 0), (0, 1), (1, 1)] + [
        (m, h) for m in range(2, n_mt) for h in range(2)
    ]

    with tile.TileContext(nc) as tc:
        with (
            tc.tile_pool(name="wpool", bufs=1) as wpool,
            tc.tile_pool(name="pspool", bufs=4, space="PSUM") as pspool,
            tc.tile_pool(name="opool", bufs=10) as opool,
        ):
            # input loads on the Scalar HWDGE queue (the Sync queue carries
            # only output stores, so input issues never FIFO-block them):
            # ux (256 KB) + vy rows 0..63 in pipelined 2048-col chunks
            # (256 KB each); rows 64..127 of each vy chunk are zeroed by
            # the otherwise-idle GpSimd engine (1.7us per chunk) instead of
            # shipping zeros through the DMA engines
            ux_t = wpool.tile([RANK, MX], bf16, name="ux_t", tag="ux_t")
            vy_t = wpool.tile([RANK, NY], bf16, name="vy_t", tag="vy_t")
            nc.scalar.dma_start(ux_t[0:NROWS, :], ux_d[:])
            nc.gpsimd.memset(ux_t[NROWS:RANK, :], 0.0)
            nc.scalar.dma_start(vy_t[0:NROWS, 0:1024], vy_d[:, 0:1024])
            nc.gpsimd.memset(vy_t[NROWS:RANK, 0:1024], 0.0)

            # PE warm-up: dummy matmuls on a zeroed tile keep the PE busy
            # while inputs stream in, so the HAM clock gate is at 2.4 GHz
            # when the real matmuls start.
            warm_t = wpool.tile([128, 512], bf16, name="warm_t", tag="warm_t")
            nc.vector.memset(warm_t[:], 0.0)
            warm_ps = pspool.tile([128, 512], f32, name="warm_ps", tag="ps")
            for _w in range(7):
                nc.tensor.matmul(
                    warm_ps[:, 0:512],
                    warm_t[:, 0:128],
                    warm_t[:],
                    start=True,
                    stop=True,
                )
            nc.scalar.dma_start(vy_t[0:NROWS, 1024:4096], vy_d[:, 1024:4096])
            nc.gpsimd.memset(vy_t[NROWS:RANK, 1024:4096], 0.0)
            nc.scalar.dma_start(vy_t[0:NROWS, 4096:8192], vy_d[:, 4096:8192])
            nc.gpsimd.memset(vy_t[NROWS:RANK, 4096:8192], 0.0)

            ti = 0
            for m, half in order:
                msl = slice(m * 128, (m + 1) * 128)
                out_t = opool.tile(
                    [128, NY // 2], u8, name=f"out_{m}_{half}", tag="out_t"
                )
                for g in range(4):
                    ps = pspool.tile(
                        [128, 1024], f32, name=f"ps_{m}_{half}_{g}", tag="ps"
                    )
                    for s in range(2):
                        col = half * 4096 + g * 1024 + s * 512
                        nc.tensor.matmul(
                            ps[:, s * 512 : (s + 1) * 512],
                            ux_t[:, msl],
                            vy_t[:, col : col + 512],
                            start=True,
                            stop=True,
                        )
                    # plain cast to uint8 (the +0.25 rounding bias is a
                    # feature row, folded into the matmul)
                    osl = slice(g * 1024, (g + 1) * 1024)
                    if sched[ti] == "act":
                        nc.scalar.copy(out_t[:, osl], ps[:])
                    else:
                        nc.vector.tensor_copy(out_t[:, osl], ps[:])
                    ti += 1
                    # last half-row: store per cast so the final DMA is
                    # only 128 KB and the tail after the last cast is short
                    if (m, half) == order[-1]:
                        dc = slice(half * 4096 + g * 1024, half * 4096 + (g + 1) * 1024)
                        nc.sync.dma_start(out_d[msl, dc], out_t[:, osl])
                # store the whole 512 KB half-row in one DMA (uint8 halves
                # the bytes, so transfer-level overhead dominates smaller
                # chunks; 4 KB per-partition descriptors stay efficient)
                if (m, half) != order[-1]:
                    dcols = slice(half * 4096, (half + 1) * 4096)
                    nc.sync.dma_start(out_d[msl, dcols], out_t[:])

    nc.compile()
    _NC_CACHE = nc
    return nc


def kernel(x: np.ndarray, y: np.ndarray) -> np.ndarray:
    global LAST_EXEC_TIME_NS, LAST_TRACE_PATH
    from concourse import bass_utils

    U, V, s = _features(x, y)
    nc = _build()

    in_maps = [
        {
            "ux": np.ascontiguousarray(U[:, i * MX : (i + 1) * MX]),
            "vy": V,
        }
        for i in range(N_CORES)
    ]
    trace = bool(os.environ.get("BESSEL_TRACE"))
    res = bass_utils.run_bass_kernel_spmd(
        nc, in_maps, core_ids=list(range(N_CORES)), trace=trace
    )
    LAST_EXEC_TIME_NS = res.exec_time_ns
    if res.instructions_and_trace is not None:
        LAST_TRACE_PATH = res.instructions_and_trace[1]
    out = np.empty((NX, NY), np.float32)
    for i in range(N_CORES):
        blk = out[i * MX : (i + 1) * MX]
        np.multiply(
            res.results[i]["out"].astype(np.float32),
            np.float32(1.0 / s),
            out=blk,
        )
    return out
